# revision 11
# baseline (speedup 1.0000x reference)
"""3-layer GCN on 8 Trainium2 NeuronCores (Bass/Tile).

Strategy (graph/data parallel over nodes):
- Core c owns nodes [c*6250, (c+1)*6250), padded to 6272 rows (49*128).
- Per layer: each core computes table = dinv * (h @ W) for its slice
  (feature-major matmul, W stationary), transposes to node-major rows,
  AllGathers the padded [50176, 64] table into HBM.
- Edges are partitioned by dst core, split into two passes by src half of
  the PADDED row space (so gather indices fit int16), grouped into
  128-dst windows and padded to 128-edge blocks on the host (uniform
  across cores so the SPMD program is identical). dma_gather fetches
  256B rows per edge; segment-sum is a TensorE matmul per 128-edge block
  against an on-chip one-hot S matrix (iota + is_equal), accumulating a
  [64,128] PSUM per dst window. Pad edges carry dst_rel=-1 -> zero S col.
- norm factorizes: norm[e] = dinv[src]*dinv[dst]; dinv[src] folds into
  the table, dinv[dst] applies at window flush. Self-loop = add the
  local dinv*table slice. GCN bias b cancels under BatchNorm. BN stats
  are reduced on-chip, AllReduced as [64,2]; scale+shift+ReLU is one
  fused ScalarE activation.
- global_mean_pool: one-hot S over graph ids per node block, matmul into
  [64, 500] PSUM, AllReduce partial sums, multiply by 1/count (host
  broadcast tensor), final FC matmul, +bias. Host transposes the [6,500]
  device output to [500, 6].
"""
import os
import numpy as np

N = 50000
E = 800000
F = 64
C = 6
G = 500
NCORES = 8
NPC = N // NCORES          # 6250 nodes per core
NWIN = (NPC + 127) // 128  # 49 dst windows per core
LASTW = NPC - 128 * (NWIN - 1)  # 106 valid cols in last window
PADN = NWIN * 128          # 6272 padded rows per core
NPAD = PADN * NCORES       # 50176 padded table rows
PHALF = NPAD // 2          # 25088: src split for int16 gather indices
EPS = 1e-5
CHUNK_BLK = 32             # gather chunk: 32 blocks = 4096 edges = 1MB

LAST_EXEC_NS = None


def _preprocess(edge_index, batch):
    src = np.asarray(edge_index[0], np.int64)
    dst = np.asarray(edge_index[1], np.int64)
    batch = np.asarray(batch, np.int64)

    deg = np.bincount(dst, minlength=N).astype(np.float32) + 1.0
    dinv = (1.0 / np.sqrt(deg)).astype(np.float32)

    # padded table row of each src node
    src_core = src // NPC
    src_pad = src_core * PADN + (src - src_core * NPC)
    pas = (src_pad >= PHALF).astype(np.int64)
    sidx = src_pad - pas * PHALF          # < 25088, int16-safe

    core = dst // NPC
    loc = dst - core * NPC
    win = loc // 128
    drel = loc - win * 128

    gid = (core * 2 + pas) * NWIN + win
    ngroups = NCORES * 2 * NWIN
    counts = np.bincount(gid, minlength=ngroups).reshape(NCORES, 2, NWIN)

    # SPMD-uniform block counts: max over cores per (pass, win)
    nblkw = np.maximum(np.ceil(counts.max(axis=0) / 128.0).astype(np.int64), 1)
    nblk_p = nblkw.sum(axis=1)
    nidx_p = nblk_p * 128

    base_blk = np.zeros((2, NWIN), np.int64)
    base_blk[:, 1:] = np.cumsum(nblkw, axis=1)[:, :-1]

    order = np.lexsort((win, pas, core))
    gid_s = gid[order]
    grp_start = np.zeros(ngroups, np.int64)
    grp_start[1:] = np.cumsum(counts.reshape(-1))[:-1]
    pos = np.arange(E, dtype=np.int64) - grp_start[gid_s]

    core_s = core[order]
    pas_s = pas[order]
    win_s = win[order]
    sidx_s = sidx[order]
    drel_s = drel[order]
    slot = base_blk[pas_s, win_s] * 128 + pos

    idx_tiles = [[None, None] for _ in range(NCORES)]
    dst_tiles = [[None, None] for _ in range(NCORES)]
    for c in range(NCORES):
        for p in range(2):
            m = (core_s == c) & (pas_s == p)
            ni = int(nidx_p[p])
            idx_arr = np.zeros(ni, np.int16)
            dr_arr = np.full(ni, -1.0, np.float32)
            idx_arr[slot[m]] = sidx_s[m].astype(np.int16)
            dr_arr[slot[m]] = drel_s[m]
            idx_tiles[c][p] = np.tile(idx_arr.reshape(-1, 16).T, (8, 1)).copy()
            dst_tiles[c][p] = dr_arr.reshape(-1, 128).T.copy()

    batchf = []
    dinv_b = []
    for c in range(NCORES):
        b = np.full(PADN, -1.0, np.float32)
        b[:NPC] = batch[c * NPC : (c + 1) * NPC].astype(np.float32)
        batchf.append(b.reshape(NWIN, 128).T.copy())
        d = dinv[c * NPC : (c + 1) * NPC]
        dinv_b.append(np.ascontiguousarray(np.tile(d[None, :], (F, 1))))

    cnts = np.bincount(batch, minlength=G).astype(np.float32)
    invcnt = (1.0 / np.maximum(cnts, 1.0)).astype(np.float32)
    invcnt_b = np.ascontiguousarray(np.tile(invcnt[None, :], (F, 1)))

    meta = dict(nblkw=nblkw, nblk_p=[int(v) for v in nblk_p],
                nidx_p=[int(v) for v in nidx_p])
    return meta, idx_tiles, dst_tiles, batchf, dinv_b, invcnt_b


def _build(meta):
    from concourse import bacc, mybir, tile
    from concourse.masks import make_identity

    nblkw = meta["nblkw"]
    nblk_p = meta["nblk_p"]
    nidx_p = meta["nidx_p"]

    nc = bacc.Bacc("TRN2", target_bir_lowering=False, debug=False,
                   num_devices=NCORES)
    dt = mybir.dt

    xT_d = nc.dram_tensor("xT", [F, NPC], dt.float32, kind="ExternalInput")
    dinvb_d = nc.dram_tensor("dinvb", [F, NPC], dt.float32, kind="ExternalInput")
    idx_d = [nc.dram_tensor(f"idx{p}", [128, nidx_p[p] // 16], dt.int16,
                            kind="ExternalInput") for p in range(2)]
    dst_d = [nc.dram_tensor(f"dstrel{p}", [128, nblk_p[p]], dt.float32,
                            kind="ExternalInput") for p in range(2)]
    batchf_d = nc.dram_tensor("batchf", [128, NWIN], dt.float32,
                              kind="ExternalInput")
    invcnt_d = nc.dram_tensor("invcntb", [F, G], dt.float32,
                              kind="ExternalInput")
    w_d = [nc.dram_tensor(f"W{l}", [F, F], dt.float32, kind="ExternalInput")
           for l in range(3)]
    g_d = [nc.dram_tensor(f"g{l}", [F, 1], dt.float32, kind="ExternalInput")
           for l in range(3)]
    be_d = [nc.dram_tensor(f"be{l}", [F, 1], dt.float32, kind="ExternalInput")
            for l in range(3)]
    fcw_d = nc.dram_tensor("fcW", [F, C], dt.float32, kind="ExternalInput")
    fcb_d = nc.dram_tensor("fcb", [C, 1], dt.float32, kind="ExternalInput")
    out_d = nc.dram_tensor("out", [C, G], dt.float32, kind="ExternalOutput")

    RG = [list(range(NCORES))]

    with tile.TileContext(nc) as tc:
        with tc.tile_pool(name="sbuf", bufs=1) as sb, \
             tc.tile_pool(name="msgs", bufs=2) as msgp, \
             tc.tile_pool(name="spool", bufs=4) as spool, \
             tc.tile_pool(name="flpool", bufs=3) as flp, \
             tc.tile_pool(name="chk", bufs=2) as chk, \
             tc.tile_pool(name="psseg", bufs=2, space="PSUM") as pseg, \
             tc.tile_pool(name="pstr", bufs=2, space="PSUM") as ptr, \
             tc.tile_pool(name="psmm", bufs=2, space="PSUM") as pmm, \
             tc.tile_pool(name="psmisc", bufs=1, space="PSUM") as pmisc, \
             tc.tile_pool(name="dram", bufs=1, space="DRAM") as dram:

            h_cur = sb.tile([F, NPC], dt.float32, tag="hcur")
            h_nxt = sb.tile([F, NPC], dt.float32, tag="hnxt")
            tableT = sb.tile([F, NPC], dt.float32, tag="tableT")
            dinvb = sb.tile([F, NPC], dt.float32, tag="dinvb")
            table_nm = sb.tile([128, NWIN, F], dt.float32, tag="tablenm")
            idx_t = [sb.tile([128, nidx_p[p] // 16], dt.int16, tag=f"idx{p}",
                             name=f"idx{p}") for p in range(2)]
            dst_t = [sb.tile([128, nblk_p[p]], dt.float32, tag=f"dst{p}",
                             name=f"dst{p}") for p in range(2)]
            iota_i = sb.tile([128, 128], dt.int32, tag="iotai")
            iota_f = sb.tile([128, 128], dt.float32, tag="iotaf")
            iotag_i = sb.tile([128, G], dt.int32, tag="iotagi")
            iotag_f = sb.tile([128, G], dt.float32, tag="iotagf")
            ident = sb.tile([F, F], dt.float32, tag="ident")
            w_t = [sb.tile([F, F], dt.float32, tag=f"w{l}", name=f"w{l}")
                   for l in range(3)]
            g_t = [sb.tile([F, 1], dt.float32, tag=f"g{l}", name=f"g{l}")
                   for l in range(3)]
            be_t = [sb.tile([F, 1], dt.float32, tag=f"be{l}", name=f"be{l}")
                    for l in range(3)]
            fcw_t = sb.tile([F, C], dt.float32, tag="fcw")
            fcb_t = sb.tile([C, 1], dt.float32, tag="fcb")
            batchf_t = sb.tile([128, NWIN], dt.float32, tag="batchf")
            invcnt_t = sb.tile([F, G], dt.float32, tag="invcnt")

            nc.sync.dma_start(out=h_cur[:], in_=xT_d[:])
            nc.sync.dma_start(out=dinvb[:], in_=dinvb_d[:])
            for p in range(2):
                nc.sync.dma_start(out=idx_t[p][:], in_=idx_d[p][:])
                nc.sync.dma_start(out=dst_t[p][:], in_=dst_d[p][:])
            nc.sync.dma_start(out=batchf_t[:], in_=batchf_d[:])
            nc.sync.dma_start(out=invcnt_t[:], in_=invcnt_d[:])
            for l in range(3):
                nc.sync.dma_start(out=w_t[l][:], in_=w_d[l][:])
                nc.sync.dma_start(out=g_t[l][:], in_=g_d[l][:])
                nc.sync.dma_start(out=be_t[l][:], in_=be_d[l][:])
            nc.sync.dma_start(out=fcw_t[:], in_=fcw_d[:])
            nc.sync.dma_start(out=fcb_t[:], in_=fcb_d[:])
            nc.gpsimd.iota(iota_i[:], pattern=[[1, 128]], base=0,
                           channel_multiplier=0)
            nc.vector.tensor_copy(iota_f[:], iota_i[:])
            nc.gpsimd.iota(iotag_i[:], pattern=[[1, G]], base=0,
                           channel_multiplier=0)
            nc.vector.tensor_copy(iotag_f[:], iotag_i[:])
            make_identity(nc, ident[:])

            def layer(l, h_in, h_out):
                # table build: tableT = ((h_in*dinv) @ W)^T, chunked
                for ci, k in enumerate(range(0, NPC, 512)):
                    ke = min(k + 512, NPC)
                    hs = chk.tile([F, 512], dt.float32, tag="hs")
                    nc.vector.tensor_tensor(
                        out=hs[:, : ke - k], in0=h_in[:, k:ke],
                        in1=dinvb[:, k:ke], op=mybir.AluOpType.mult)
                    psm = pmm.tile([F, 512], dt.float32, space="PSUM",
                                   tag="psmm")
                    nc.tensor.matmul(out=psm[:, : ke - k], lhsT=w_t[l][:],
                                     rhs=hs[:, : ke - k], start=True,
                                     stop=True)
                    nc.vector.tensor_copy(tableT[:, k:ke], psm[:, : ke - k])
                # transpose to node-major + stage + AllGather
                for w in range(NWIN):
                    k = w * 128
                    cw = 128 if w < NWIN - 1 else LASTW
                    pst = ptr.tile([128, F], dt.float32, space="PSUM",
                                   tag="pstr")
                    nc.tensor.transpose(out=pst[:cw, :],
                                        in_=tableT[:, k : k + cw],
                                        identity=ident[:])
                    if cw < 128:
                        nc.vector.memset(table_nm[:, w, :], 0.0)
                    nc.vector.tensor_copy(table_nm[:cw, w, :], pst[:cw, :])
                ag_in = dram.tile([PADN, F], dt.float32, tag=f"agin{l}")
                table_full = dram.tile([NPAD, F], dt.float32,
                                       addr_space="Shared", tag=f"tfull{l}")
                nc.sync.dma_start(
                    out=ag_in[:].rearrange("(w p) f -> p w f", p=128),
                    in_=table_nm[:],
                )
                nc.gpsimd.collective_compute(
                    "AllGather", mybir.AluOpType.bypass, replica_groups=RG,
                    ins=[ag_in[:]], outs=[table_full[:]],
                )
                # self-loop term: tableT *= dinv (in place)
                nc.vector.tensor_tensor(out=tableT[:], in0=tableT[:],
                                        in1=dinvb[:], op=mybir.AluOpType.mult)

                # gather + segment-sum; x1 written into h_out pre-BN
                halves = [table_full[0:PHALF, :], table_full[PHALF:NPAD, :]]
                chunk_tiles = {}

                def get_chunk(p, b):
                    ck = b // CHUNK_BLK
                    key = (p, ck)
                    if key not in chunk_tiles:
                        nb = min(CHUNK_BLK, nblk_p[p] - ck * CHUNK_BLK)
                        t = msgp.tile([128, CHUNK_BLK, F], dt.float32,
                                      tag=f"msgs{p}", name=f"msgs{p}")
                        col0 = ck * CHUNK_BLK * 8
                        nc.gpsimd.dma_gather(
                            out_ap=t[:, :nb, :],
                            in_ap=halves[p],
                            idxs_ap=idx_t[p][:, col0 : col0 + nb * 8],
                            num_idxs=nb * 128,
                            num_idxs_reg=nb * 128,
                            elem_size=F,
                            single_packet=False,
                        )
                        chunk_tiles[key] = (t, ck * CHUNK_BLK)
                    return chunk_tiles[key]

                bpos = [0, 0]
                for w in range(NWIN):
                    k = w * 128
                    cw = 128 if w < NWIN - 1 else LASTW
                    total = int(nblkw[0][w] + nblkw[1][w])
                    ps = pseg.tile([F, 128], dt.float32, space="PSUM",
                                   tag="pseg")
                    cnt = 0
                    for p in range(2):
                        for _ in range(int(nblkw[p][w])):
                            b = bpos[p]
                            bpos[p] += 1
                            mt, base = get_chunk(p, b)
                            s_t = spool.tile([128, 128], dt.float32, tag="s")
                            nc.vector.tensor_tensor(
                                out=s_t[:],
                                in0=dst_t[p][:, b : b + 1].to_broadcast(
                                    [128, 128]),
                                in1=iota_f[:],
                                op=mybir.AluOpType.is_equal,
                            )
                            nc.tensor.matmul(
                                out=ps[:], lhsT=mt[:, b - base, :],
                                rhs=s_t[:], start=(cnt == 0),
                                stop=(cnt == total - 1),
                            )
                            cnt += 1
                    fl = flp.tile([F, 128], dt.float32, tag="fl")
                    nc.vector.tensor_tensor(
                        out=fl[:, :cw], in0=ps[:, :cw],
                        in1=dinvb[:, k : k + cw], op=mybir.AluOpType.mult)
                    nc.vector.tensor_tensor(
                        out=h_out[:, k : k + cw], in0=fl[:, :cw],
                        in1=tableT[:, k : k + cw], op=mybir.AluOpType.add)

                # BN stats (sum, sumsq) -> AllReduce
                nchk = (NPC + 511) // 512
                stats = sb.tile([F, 2], dt.float32, tag="stats")
                scol = sb.tile([F, nchk], dt.float32, tag="scol")
                for ci, k in enumerate(range(0, NPC, 512)):
                    ke = min(k + 512, NPC)
                    sq = chk.tile([F, 512], dt.float32, tag="sq")
                    nc.scalar.activation(
                        sq[:, : ke - k], h_out[:, k:ke],
                        mybir.ActivationFunctionType.Square,
                        accum_out=scol[:, ci : ci + 1])
                nc.vector.tensor_reduce(
                    out=stats[:, 1:2], in_=scol[:], axis=mybir.AxisListType.X,
                    op=mybir.AluOpType.add)
                nc.vector.tensor_reduce(
                    out=stats[:, 0:1], in_=h_out[:], axis=mybir.AxisListType.X,
                    op=mybir.AluOpType.add)
                ar_in = dram.tile([F, 2], dt.float32, tag=f"arin{l}")
                ar_out = dram.tile([F, 2], dt.float32, addr_space="Shared",
                                   tag=f"arout{l}")
                nc.sync.dma_start(out=ar_in[:], in_=stats[:])
                nc.gpsimd.collective_compute(
                    "AllReduce", mybir.AluOpType.add, replica_groups=RG,
                    ins=[ar_in[:]], outs=[ar_out[:]],
                )
                statg = sb.tile([F, 2], dt.float32, tag="statg")
                nc.sync.dma_start(out=statg[:], in_=ar_out[:])

                # scale/bias; h_out = relu(h_out*scale + bias) in place
                mu = sb.tile([F, 1], dt.float32, tag="mu")
                var = sb.tile([F, 1], dt.float32, tag="var")
                rstd = sb.tile([F, 1], dt.float32, tag="rstd")
                scale = sb.tile([F, 1], dt.float32, tag="scale")
                bias = sb.tile([F, 1], dt.float32, tag="bias")
                tmp = sb.tile([F, 1], dt.float32, tag="tmp1")
                inv_n = float(1.0 / N)
                nc.vector.tensor_scalar_mul(mu[:], statg[:, 0:1], inv_n)
                nc.vector.tensor_scalar_mul(var[:], statg[:, 1:2], inv_n)
                nc.vector.tensor_tensor(out=tmp[:], in0=mu[:], in1=mu[:],
                                        op=mybir.AluOpType.mult)
                nc.vector.tensor_tensor(out=var[:], in0=var[:], in1=tmp[:],
                                        op=mybir.AluOpType.subtract)
                std = sb.tile([F, 1], dt.float32, tag="std")
                nc.vector.tensor_scalar_add(var[:], var[:], float(EPS))
                nc.scalar.activation(std[:], var[:],
                                     mybir.ActivationFunctionType.Sqrt)
                nc.vector.reciprocal(rstd[:], std[:])
                nc.vector.tensor_tensor(out=scale[:], in0=rstd[:],
                                        in1=g_t[l][:], op=mybir.AluOpType.mult)
                nc.vector.tensor_tensor(out=tmp[:], in0=mu[:], in1=scale[:],
                                        op=mybir.AluOpType.mult)
                nc.vector.tensor_tensor(out=bias[:], in0=be_t[l][:],
                                        in1=tmp[:],
                                        op=mybir.AluOpType.subtract)
                nc.scalar.activation(h_out[:], h_out[:],
                                     mybir.ActivationFunctionType.Relu,
                                     bias=bias[:], scale=scale[:])

            layer(0, h_cur, h_nxt)
            layer(1, h_nxt, h_cur)
            layer(2, h_cur, h_nxt)

            # pooling: transpose h3 to node-major (into table_nm), then
            # one-hot matmuls accumulate [F, G] PSUM
            for w in range(NWIN):
                k = w * 128
                cw = 128 if w < NWIN - 1 else LASTW
                pst = ptr.tile([128, F], dt.float32, space="PSUM", tag="pstr")
                nc.tensor.transpose(out=pst[:cw, :], in_=h_nxt[:, k : k + cw],
                                    identity=ident[:])
                if cw < 128:
                    nc.vector.memset(table_nm[:, w, :], 0.0)
                nc.vector.tensor_copy(table_nm[:cw, w, :], pst[:cw, :])
            pspool = pmisc.tile([F, G], dt.float32, space="PSUM", tag="pspool")
            for w in range(NWIN):
                sp = spool.tile([128, G], dt.float32, tag="spg")
                nc.vector.tensor_tensor(
                    out=sp[:],
                    in0=batchf_t[:, w : w + 1].to_broadcast([128, G]),
                    in1=iotag_f[:], op=mybir.AluOpType.is_equal)
                nc.tensor.matmul(out=pspool[:], lhsT=table_nm[:, w, :],
                                 rhs=sp[:], start=(w == 0),
                                 stop=(w == NWIN - 1))
            pooled = sb.tile([F, G], dt.float32, tag="pooled")
            nc.vector.tensor_copy(pooled[:], pspool[:])
            pl_in = dram.tile([F, G], dt.float32, tag="plin")
            pl_out = dram.tile([F, G], dt.float32, addr_space="Shared",
                               tag="plout")
            nc.sync.dma_start(out=pl_in[:], in_=pooled[:])
            nc.gpsimd.collective_compute(
                "AllReduce", mybir.AluOpType.add, replica_groups=RG,
                ins=[pl_in[:]], outs=[pl_out[:]],
            )
            pooled_g = sb.tile([F, G], dt.float32, tag="pooledg")
            nc.sync.dma_start(out=pooled_g[:], in_=pl_out[:])
            nc.vector.tensor_tensor(out=pooled_g[:], in0=pooled_g[:],
                                    in1=invcnt_t[:], op=mybir.AluOpType.mult)
            psfc = pmisc.tile([C, G], dt.float32, space="PSUM", tag="psfc")
            nc.tensor.matmul(out=psfc[:], lhsT=fcw_t[:], rhs=pooled_g[:],
                             start=True, stop=True)
            out_t = sb.tile([C, G], dt.float32, tag="outt")
            nc.vector.tensor_scalar_add(out_t[:], psfc[:], fcb_t[:])
            nc.sync.dma_start(out=out_d[:], in_=out_t[:])

    nc.compile()
    return nc


def _install_trace_shim():
    """Provide antenv.axon_hooks (missing in this image) so the NTFF
    trace path of run_bass_kernel_spmd works; defuse artifact upload."""
    import sys, types
    try:
        import antenv.axon_hooks  # noqa: F401
        ok = True
    except ImportError:
        ok = False
    if not ok:
        try:
            from trn_agent_boot.trn_boot import _ntff_profile_via_ctypes
        except ImportError:
            return False
        hook = _ntff_profile_via_ctypes("/opt/axon/libaxon_pjrt.so")
        if hook is None:
            return False
        mod = types.ModuleType("antenv.axon_hooks")
        mod.get_axon_ntff_profile_hook = lambda: hook
        mod.set_axon_ntff_profile_hook = lambda h: None
        sys.modules["antenv.axon_hooks"] = mod
    import concourse.bass_utils as bu
    bu.upload_artifacts = lambda tmpdir: str(tmpdir)
    return True


def _ensure_axon():
    """run_bass_via_pjrt uses jax.devices(); make sure axon is default."""
    import jax
    try:
        if jax.devices()[0].platform == "axon":
            return
    except Exception:
        pass
    jax.config.update("jax_platforms", "axon")
    try:
        jax.extend.backend.clear_backends()
    except Exception:
        pass


def kernel(x, edge_index, batch, W0, b0, g0, be0, W1, b1, g1, be1,
           W2, b2, g2, be2, fcW, fcb):
    global LAST_EXEC_NS
    _ensure_axon()
    from concourse.bass_utils import run_bass_kernel_spmd

    x = np.ascontiguousarray(np.asarray(x, np.float32))
    meta, idx_tiles, dst_tiles, batchf, dinv_b, invcnt_b = _preprocess(
        edge_index, batch)
    nc = _build(meta)

    Ws = [np.ascontiguousarray(np.asarray(w, np.float32)) for w in (W0, W1, W2)]
    gs = [np.asarray(v, np.float32).reshape(F, 1).copy() for v in (g0, g1, g2)]
    bes = [np.asarray(v, np.float32).reshape(F, 1).copy()
           for v in (be0, be1, be2)]
    fcW = np.ascontiguousarray(np.asarray(fcW, np.float32))
    fcb = np.asarray(fcb, np.float32).reshape(C, 1).copy()

    in_maps = []
    for c in range(NCORES):
        m = {
            "xT": np.ascontiguousarray(x[c * NPC : (c + 1) * NPC].T),
            "dinvb": dinv_b[c],
            "batchf": batchf[c],
            "invcntb": invcnt_b,
            "fcW": fcW, "fcb": fcb,
        }
        for p in range(2):
            m[f"idx{p}"] = idx_tiles[c][p]
            m[f"dstrel{p}"] = dst_tiles[c][p]
        for l in range(3):
            m[f"W{l}"] = Ws[l]
            m[f"g{l}"] = gs[l]
            m[f"be{l}"] = bes[l]
        in_maps.append(m)

    trace = os.environ.get("GCN_TRACE", "0") == "1"
    tmpdir = None
    if trace:
        trace = _install_trace_shim()
        if trace:
            import tempfile
            tmpdir = tempfile.mkdtemp(prefix="gcn_trace_")
            print("trace dir:", tmpdir)
    res = run_bass_kernel_spmd(nc, in_maps, core_ids=list(range(NCORES)),
                               trace=trace, tmpdir=tmpdir)
    LAST_EXEC_NS = res.exec_time_ns
    out = res.results[0]["out"]
    return np.ascontiguousarray(out.T).astype(np.float32)
